# revision 13
# baseline (speedup 1.0000x reference)
"""Trainium2 Bass kernel: batched truncated matrix exponential of
skew-symmetrized 256x256 matrices (nn_BatchedExponentialOrthogonalization).

Full input:  w   [512, 256, 256] fp32
Full output: out [512, 256, 256] fp32
  a = (w - w^T)/2 per matrix;  out = I + a + a^2/2! + ... + a^6/6!

Sharding: leading batch dim split across 8 NeuronCores (64 matrices each),
fully data-parallel (SPMD, same NEFF, different slabs).

Math (Horner in a^2; exactly 3 matmuls of 256^3 per matrix, the PE minimum
for a degree-6 polynomial, and NO extra identity-patch matmuls).
The host sends Aid = (w - w^T) + 6I and Ap = w - w^T in fp16 (the skew
diag is exactly 0, so Aid's diagonal is exactly 6.0).  With s = a^2:
  P1  = Ap^T Ap = -4s                  (mm1; Ap^T = -Ap)
  Bq  = -P1/12 = s/3                   (ACT psum->SBUF scaled copy)
  T1  = Aid + Bq = 2a + 6I + s/3       (tensor_tensor, DVE)
  U   = Bq . T1 = (2/3)sa + 2s + s^2/9 (mm2; Bq symmetric)
  X2  = 0.6*Bq + Aid = 0.2s + 2a + 6I  (DVE: 4x tensor_scalar + 2x tt)
  T2  = 0.15*U + X2
      = s^2/60 + 0.1 sa + 0.5 s + 2a + 6I   (DVE stt, psum in0)
  V   = Bq . T2                        (mm3)
  out = V/4 = s/2 + sa/6 + s^2/24 + s^2 a/120 + s^3/720   (ACT, fp16)
All coefficients are EXACT (no a- or I-contamination in out); the host
adds back I + a in fp32: res = out + (w - w^T)/2 (+1 on the diag).

Precision: fp16 operands (psum fp32), fp16 output; I + a exact fp32 on
host.  Rel err ~5e-4 (tolerance 2e-2).

Engines per pair-step (2 matrices): PE 24 matmuls (the minimum); DVE
T1/X2/T2 (GPSIMD fully idle - it shares SBUF ports with the DVE and
any concurrent GPSIMD op ~2.5x-slows DVE ops, so the diag-zeroed Ap
comes from the host instead of an on-device affine_select); ACT Bq +
V/4; DMA fp16 in (Aid + Ap) + fp16 out (no XBAR transpose).
Emission order inside a stage keeps the PSUM WARs loose: ACT does the
V/4 copy before Bq (frees vp's slot), DVE does T2 first (frees up's
slot before the same-stage mm3 rotates into it).  PSUM is split into
{bp,vp} / {warm,up} pools of 2 bufs each (8 banks exactly).  Stores go
per-pair from the GPSIMD queue; input b streams before a with group 0
quartered, overlapping the PE warm-up matmuls.
"""
from contextlib import ExitStack

import numpy as np

import concourse.bass as bass
import concourse.mybir as mybir
import concourse.tile as tile
from concourse.bass_utils import run_bass_kernel_spmd

F32 = mybir.dt.float32
F16 = mybir.dt.float16
N = 256
H = 128
N_CORES = 8
N_MAT_PER_CORE = 64
GROUP = 8
_MAX_WAITS = 1

N_WARM = 6


def _split_multi_waits(nc, max_waits=_MAX_WAITS):
    """This container's walrus accepts at most one sync wait per
    instruction; move excess waits onto no-fuse NOPs inserted immediately
    before, on the same engine (semantically identical - engines execute
    their stream serially)."""
    for f in nc.m.functions:
        for b in f.blocks:
            insts = b.instructions
            if not any(
                i.sync_info and i.sync_info.on_wait
                and len(i.sync_info.on_wait) > max_waits
                for i in insts
            ):
                continue
            new = []
            for inst in insts:
                si = inst.sync_info
                if si and si.on_wait and len(si.on_wait) > max_waits:
                    waits = list(si.on_wait)
                    extra, keep = waits[:-max_waits], waits[-max_waits:]
                    for k in range(0, len(extra), max_waits):
                        nop = mybir.InstNoOp(
                            name=f"I-waitsplit-{nc.next_id()}", ins=[], outs=[])
                        nop.engine = inst.engine
                        nop.bass_nofuse = True
                        nop.sync_info = mybir.SyncInfo(
                            on_wait=extra[k:k + max_waits], on_update=[])
                        new.append(nop)
                    inst.sync_info = mybir.SyncInfo(
                        on_wait=keep, on_update=list(si.on_update or []))
                new.append(inst)
            insts.clear()
            insts.extend(new)


def _build_kernel(n_mat=N_MAT_PER_CORE, group=GROUP, split_waits=True):
    nc = bass.Bass(trn_type="TRN2")
    a_in = nc.dram_tensor("a", [n_mat, N, N], F16, kind="ExternalInput")
    b_in = nc.dram_tensor("b", [n_mat, N, N], F16, kind="ExternalInput")
    out = nc.dram_tensor("out", [n_mat, N, N], F16, kind="ExternalOutput")
    n_groups = n_mat // group
    n_pairs = n_mat // 2
    PPG = group // 2  # pairs per group

    mult = mybir.AluOpType.mult
    add = mybir.AluOpType.add

    with ExitStack() as ctx:
        tc = ctx.enter_context(tile.TileContext(nc))
        const_pool = ctx.enter_context(tc.tile_pool(name="const", bufs=1))
        io_pool = ctx.enter_context(tc.tile_pool(name="io", bufs=8))
        bq_pool = ctx.enter_context(tc.tile_pool(name="bq", bufs=7))
        t12_pool = ctx.enter_context(tc.tile_pool(name="t12", bufs=8))
        x2_pool = ctx.enter_context(tc.tile_pool(name="x2", bufs=6))
        out_pool = ctx.enter_context(tc.tile_pool(name="outp", bufs=3))
        # PSUM split so every WAR clears >= a full stage (or an early-ACT
        # op) before the PE rotates into the slot: bp+vp alternate in ps_a
        # (bp: slot freed by the Bq ACT copy one stage earlier; vp: freed
        # by the V/4 copy that runs FIRST in the ACT stream of the same
        # stage, ~1.1us before mm3 needs it), warm+up rotate in ps_b (up's
        # slot freed by T2, a full stage earlier).
        ps_a = ctx.enter_context(
            tc.tile_pool(name="psa", bufs=2, space="PSUM"))
        ps_b = ctx.enter_context(
            tc.tile_pool(name="psb", bufs=2, space="PSUM"))

        # ---- PE p-state warm-up + ACT table preload during first DMA ----
        wz = const_pool.tile([H, 2 * N], F16, tag="wz")
        nc.vector.memset(wz[:], 0.0)
        warm = ps_b.tile([H, 2 * 2 * N], F32, tag="psb")
        for _ in range(N_WARM):
            nc.tensor.matmul(warm[:, :2 * N], wz[:, :H], wz[:, :2 * N],
                             start=True, stop=True)
        warm_sb = const_pool.tile([H, 8], F32, tag="warmsb")
        nc.scalar.copy(warm_sb[:], warm[:, 0:8])

        def load_group(g):
            # b (mm1's operand) is issued before a; group 0's b arrives in
            # pair-sized quarters so mm1(0) can start as early as possible.
            ain = io_pool.tile([H, group * 2 * N], F16, tag="ain")
            apn = io_pool.tile([H, group * 2 * N], F16, tag="apn")
            hg = group // 2
            nb = 4 if g == 0 else 2
            cb = group // nb
            for chunk in range(nb):
                m0 = chunk * cb
                nc.sync.dma_start(
                    apn[:, m0 * 2 * N:(m0 + cb) * 2 * N],
                    bass.AP(b_in, (g * group + m0) * N * N,
                            [[N, H], [N * N, cb], [H * N, 2], [1, N]]))
            for half in range(2):
                m0 = half * hg
                # a-loads issue from the GPSIMD queue (descriptor-gen only,
                # no SBUF-port contention) so Sync streams b back-to-back
                nc.gpsimd.dma_start(
                    ain[:, m0 * 2 * N:(m0 + hg) * 2 * N],
                    bass.AP(a_in, (g * group + m0) * N * N,
                            [[N, H], [N * N, hg], [H * N, 2], [1, N]]))
            return ain, apn

        def pair_view(ain, j):
            return ain[:, (2 * j) * 2 * N:(2 * j + 2) * 2 * N]

        def mm_pair(psum, lhs_tile, rhs_tile, loff=0):
            # psum[m] = M . X per matrix; lhsT blocks (k,i) at m*512+k*256+
            # i*128, rhs row-blocks k at m*512+k*256.
            for h in range(2):
                for i in range(2):
                    for k in range(2):
                        nc.tensor.matmul(
                            psum[:, h * 2 * N + i * N:h * 2 * N + (i + 1) * N],
                            lhs_tile[:, loff + h * 2 * N + k * N + i * H:
                                     loff + h * 2 * N + k * N + (i + 1) * H],
                            rhs_tile[:, h * 2 * N + k * N:
                                     h * 2 * N + (k + 1) * N],
                            start=(k == 0), stop=(k == 1))

        def mm1(apn, j):
            bp = ps_a.tile([H, 2 * 2 * N], F32, tag="psa")
            pv = pair_view(apn, j)
            mm_pair(bp, pv, pv)
            return bp

        def bq_op(bp):
            bq = bq_pool.tile([H, 2 * 2 * N], F16, tag="bq")
            nc.scalar.mul(bq[:], bp[:], -1.0 / 12.0)
            return bq

        def t1_op(ain, j, bq):
            t1 = t12_pool.tile([H, 2 * 2 * N], F16, tag="t1")
            nc.vector.tensor_tensor(t1[:], pair_view(ain, j), bq[:], op=add)
            return t1

        def x2_op(bq, ain, j):
            # 0.6*Bq via single-src tensor_scalar (4x perf mode), then a
            # plain TT (2x mode) - together ~1000ns vs a 1x-only STT ~1210ns.
            b6 = x2_pool.tile([H, 2 * 2 * N], F16, tag="b6")
            nc.vector.tensor_scalar_mul(b6[:], bq[:], 0.6)
            x2 = x2_pool.tile([H, 2 * 2 * N], F16, tag="x2")
            nc.vector.tensor_tensor(x2[:], b6[:], pair_view(ain, j), op=add)
            return x2

        def mm2(bq, t1):
            up = ps_b.tile([H, 2 * 2 * N], F32, tag="psb")
            mm_pair(up, bq, t1)
            return up

        def t2_op(up, x2):
            t2 = t12_pool.tile([H, 2 * 2 * N], F16, tag="t2")
            nc.vector.scalar_tensor_tensor(
                t2[:], up[:], 0.15, x2[:], op0=mult, op1=add)
            return t2

        def mm3(bq, t2):
            vp = ps_a.tile([H, 2 * 2 * N], F32, tag="psa")
            mm_pair(vp, bq, t2)
            return vp

        def out_copy(vp):
            wout = out_pool.tile([H, 2 * 2 * N], F16, tag="wout")
            nc.scalar.mul(wout[:], vp[:], 0.25)
            return wout

        def store_pair(p, wout):
            # issued from the (otherwise idle) GPSIMD queue to keep the
            # Sync engine's descriptor-generation budget for input loads
            nc.gpsimd.dma_start(
                bass.AP(out, 2 * p * N * N,
                        [[N, H], [N * N, 2], [H * N, 2], [1, N]]),
                wout[:])

        # ---- software-pipelined emission ----
        ain_t, apn_t = {}, {}
        bq_t, t1_t, x2_t, t2_t = {}, {}, {}, {}
        bp_t, up_t, vp_t, wout_t = {}, {}, {}, {}

        ain_t[0], apn_t[0] = load_group(0)
        if n_groups > 1:
            ain_t[1], apn_t[1] = load_group(1)

        for s in range(n_pairs + 12):
            g = s // PPG
            if s % PPG == 0 and g + 2 < n_groups:
                ain_t[g + 2], apn_t[g + 2] = load_group(g + 2)
            if s == 3:
                # keep the PE busy through the fill-cascade hole (mm2(0)
                # waits the cold mm1(0)->Bq->T1 chain ~2us): more warm
                # matmuls, before mm2(0) in the stream. Also keeps the
                # HAM activity window hot. Zero steady-state cost.
                for _ in range(7):
                    nc.tensor.matmul(warm[:, :2 * N], wz[:, :H],
                                     wz[:, :2 * N], start=True, stop=True)
            p = s - 1  # mm1
            if 0 <= p < n_pairs:
                gp, j = divmod(p, PPG)
                bp_t[p] = mm1(apn_t[gp], j)
                if j == PPG - 1:
                    apn_t.pop(gp, None)
            p = s - 3  # mm2
            if 0 <= p < n_pairs:
                up_t[p] = mm2(bq_t[p], t1_t.pop(p))
            p = s - 5  # mm3
            if 0 <= p < n_pairs:
                vp_t[p] = mm3(bq_t.pop(p), t2_t.pop(p))
            p = s - 6  # out copy (ACT) - before Bq so vp's slot frees early
            if 0 <= p < n_pairs:
                wout_t[p] = out_copy(vp_t.pop(p))
            p = s - 1  # Bq (ACT)
            if 0 <= p < n_pairs:
                bq_t[p] = bq_op(bp_t.pop(p))
            p = s - 4  # T2 (DVE) - first on DVE so up's slot frees early
            if 0 <= p < n_pairs:
                t2_t[p] = t2_op(up_t.pop(p), x2_t.pop(p))
            p = s - 2  # T1, X2 (DVE)
            if 0 <= p < n_pairs:
                gp, j = divmod(p, PPG)
                t1_t[p] = t1_op(ain_t[gp], j, bq_t[p])
                x2_t[p] = x2_op(bq_t[p], ain_t[gp], j)
                if j == PPG - 1:
                    ain_t.pop(gp, None)
            # pair store, two steps after its out_copy
            p = s - 8
            if 0 <= p < n_pairs:
                store_pair(p, wout_t.pop(p))

    if split_waits:
        _split_multi_waits(nc)
    return nc


_NC_CACHE = {}


def _prep_input(w: np.ndarray):
    """Aid = (w - w^T) + 6I and Ap = w - w^T, fp16 (skew diag exactly 0)."""
    ap = w - np.swapaxes(w, -1, -2)
    ap16 = np.ascontiguousarray(ap.astype(np.float16))
    idx = np.arange(N)
    ap[:, idx, idx] = 6.0
    aid16 = np.ascontiguousarray(ap.astype(np.float16))
    return aid16, ap16


def _postprocess(raw: np.ndarray, w: np.ndarray) -> np.ndarray:
    """res = raw (device series terms, fp16) + (w - w^T)/2 + I, in fp32."""
    res = raw.astype(np.float32)
    res += (w - np.swapaxes(w, -1, -2)) * 0.5
    idx = np.arange(N)
    res[:, idx, idx] += 1.0
    return res


def kernel(w: np.ndarray) -> np.ndarray:
    w = np.ascontiguousarray(np.asarray(w, dtype=np.float32))
    n_total = w.shape[0]
    assert w.shape == (n_total, N, N)
    per = n_total // N_CORES
    if per not in _NC_CACHE:
        _NC_CACHE[per] = _build_kernel(n_mat=per)
    nc = _NC_CACHE[per]
    aid16, ap16 = _prep_input(w)
    in_maps = [{"a": aid16[i * per:(i + 1) * per],
                "b": ap16[i * per:(i + 1) * per]} for i in range(N_CORES)]
    res = run_bass_kernel_spmd(nc, in_maps, core_ids=list(range(N_CORES)))
    raw = np.concatenate(
        [np.asarray(r["out"]).astype(np.float32) for r in res.results],
        axis=0)
    return _postprocess(raw, w)


# revision 14
# speedup vs baseline: 1.3319x; 1.3319x over previous
"""Trainium2 Bass kernel: batched truncated matrix exponential of
skew-symmetrized 256x256 matrices (nn_BatchedExponentialOrthogonalization).

Full input:  w   [512, 256, 256] fp32
Full output: out [512, 256, 256] fp32
  a = (w - w^T)/2 per matrix;  out = I + a + a^2/2! + ... + a^6/6!

Sharding: leading batch dim split across 8 NeuronCores (64 matrices each),
fully data-parallel (SPMD, same NEFF, different slabs).

Math (Horner in a^2; exactly 3 matmuls of 256^3 per matrix, the PE minimum
for a degree-6 polynomial, and NO extra identity-patch matmuls).
The host sends Aid = (w - w^T) + 6I and Ap = w - w^T in fp16 (the skew
diag is exactly 0, so Aid's diagonal is exactly 6.0).  With s = a^2:
  P1  = Ap^T Ap = -4s                  (mm1; Ap^T = -Ap)
  Bq  = -P1/12 = s/3                   (ACT psum->SBUF scaled copy)
  T1  = Aid + Bq = 2a + 6I + s/3       (tensor_tensor, DVE)
  U   = Bq . T1 = (2/3)sa + 2s + s^2/9 (mm2; Bq symmetric)
  X2  = 0.6*Bq + Aid = 0.2s + 2a + 6I  (DVE: 4x tensor_scalar + 2x tt)
  T2  = 0.15*U + X2
      = s^2/60 + 0.1 sa + 0.5 s + 2a + 6I   (DVE stt, psum in0)
  V   = Bq . T2                        (mm3)
  out = V/4 = s/2 + sa/6 + s^2/24 + s^2 a/120 + s^3/720   (ACT, fp16)
All coefficients are EXACT (no a- or I-contamination in out); the host
adds back I + a in fp32: res = out + (w - w^T)/2 (+1 on the diag).

Precision: fp16 operands (psum fp32), fp16 output; I + a exact fp32 on
host.  Rel err ~5e-4 (tolerance 2e-2).

Engines per pair-step (2 matrices): PE 24 matmuls (the minimum); DVE
T1/X2/T2 (GPSIMD fully idle - it shares SBUF ports with the DVE and
any concurrent GPSIMD op ~2.5x-slows DVE ops, so the diag-zeroed Ap
comes from the host instead of an on-device affine_select); ACT Bq +
V/4; DMA fp16 in (Aid + Ap) + fp16 out (no XBAR transpose).
Emission order inside a stage keeps the PSUM WARs loose: ACT does the
V/4 copy before Bq (frees vp's slot), DVE does T2 first (frees up's
slot before the same-stage mm3 rotates into it).  PSUM is split into
{bp,vp} / {warm,up} pools of 2 bufs each (8 banks exactly).  Stores go
per-pair from the GPSIMD queue; input b streams before a with group 0
quartered, overlapping the PE warm-up matmuls.
"""
from contextlib import ExitStack

import numpy as np

import concourse.bass as bass
import concourse.mybir as mybir
import concourse.tile as tile
from concourse.bass_utils import run_bass_kernel_spmd

F32 = mybir.dt.float32
F16 = mybir.dt.float16
N = 256
H = 128
N_CORES = 8
N_MAT_PER_CORE = 64
GROUP = 8
_MAX_WAITS = 1

N_WARM = 6


def _split_multi_waits(nc, max_waits=_MAX_WAITS):
    """This container's walrus accepts at most one sync wait per
    instruction; move excess waits onto no-fuse NOPs inserted immediately
    before, on the same engine (semantically identical - engines execute
    their stream serially)."""
    for f in nc.m.functions:
        for b in f.blocks:
            insts = b.instructions
            if not any(
                i.sync_info and i.sync_info.on_wait
                and len(i.sync_info.on_wait) > max_waits
                for i in insts
            ):
                continue
            new = []
            for inst in insts:
                si = inst.sync_info
                if si and si.on_wait and len(si.on_wait) > max_waits:
                    waits = list(si.on_wait)
                    extra, keep = waits[:-max_waits], waits[-max_waits:]
                    for k in range(0, len(extra), max_waits):
                        nop = mybir.InstNoOp(
                            name=f"I-waitsplit-{nc.next_id()}", ins=[], outs=[])
                        nop.engine = inst.engine
                        nop.bass_nofuse = True
                        nop.sync_info = mybir.SyncInfo(
                            on_wait=extra[k:k + max_waits], on_update=[])
                        new.append(nop)
                    inst.sync_info = mybir.SyncInfo(
                        on_wait=keep, on_update=list(si.on_update or []))
                new.append(inst)
            insts.clear()
            insts.extend(new)


def _build_kernel(n_mat=N_MAT_PER_CORE, group=GROUP, split_waits=True):
    nc = bass.Bass(trn_type="TRN2")
    a_in = nc.dram_tensor("a", [n_mat, N, N], F16, kind="ExternalInput")
    b_in = nc.dram_tensor("b", [n_mat, N, N], F16, kind="ExternalInput")
    out = nc.dram_tensor("out", [n_mat, N, N], F16, kind="ExternalOutput")
    n_groups = n_mat // group
    n_pairs = n_mat // 2
    PPG = group // 2  # pairs per group

    mult = mybir.AluOpType.mult
    add = mybir.AluOpType.add

    with ExitStack() as ctx:
        tc = ctx.enter_context(tile.TileContext(nc))
        const_pool = ctx.enter_context(tc.tile_pool(name="const", bufs=1))
        io_pool = ctx.enter_context(tc.tile_pool(name="io", bufs=8))
        bq_pool = ctx.enter_context(tc.tile_pool(name="bq", bufs=7))
        t12_pool = ctx.enter_context(tc.tile_pool(name="t12", bufs=8))
        x2_pool = ctx.enter_context(tc.tile_pool(name="x2", bufs=6))
        out_pool = ctx.enter_context(tc.tile_pool(name="outp", bufs=3))
        # PSUM split so every WAR clears >= a full stage (or an early-ACT
        # op) before the PE rotates into the slot: bp+vp alternate in ps_a
        # (bp: slot freed by the Bq ACT copy one stage earlier; vp: freed
        # by the V/4 copy that runs FIRST in the ACT stream of the same
        # stage, ~1.1us before mm3 needs it), warm+up rotate in ps_b (up's
        # slot freed by T2, a full stage earlier).
        ps_a = ctx.enter_context(
            tc.tile_pool(name="psa", bufs=2, space="PSUM"))
        ps_b = ctx.enter_context(
            tc.tile_pool(name="psb", bufs=2, space="PSUM"))

        # ---- PE p-state warm-up + ACT table preload during first DMA ----
        wz = const_pool.tile([H, 2 * N], F16, tag="wz")
        nc.vector.memset(wz[:], 0.0)
        warm = ps_b.tile([H, 2 * 2 * N], F32, tag="psb")
        for _ in range(N_WARM):
            nc.tensor.matmul(warm[:, :2 * N], wz[:, :H], wz[:, :2 * N],
                             start=True, stop=True)
        warm_sb = const_pool.tile([H, 8], F32, tag="warmsb")
        nc.scalar.copy(warm_sb[:], warm[:, 0:8])

        def load_group(g):
            # b (mm1's operand) is issued before a; group 0's b arrives in
            # pair-sized quarters so mm1(0) can start as early as possible.
            ain = io_pool.tile([H, group * 2 * N], F16, tag="ain")
            apn = io_pool.tile([H, group * 2 * N], F16, tag="apn")
            hg = group // 2
            nb = 4 if g == 0 else 2
            cb = group // nb
            for chunk in range(nb):
                m0 = chunk * cb
                nc.sync.dma_start(
                    apn[:, m0 * 2 * N:(m0 + cb) * 2 * N],
                    bass.AP(b_in, (g * group + m0) * N * N,
                            [[N, H], [N * N, cb], [H * N, 2], [1, N]]))
            for half in range(2):
                m0 = half * hg
                nc.sync.dma_start(
                    ain[:, m0 * 2 * N:(m0 + hg) * 2 * N],
                    bass.AP(a_in, (g * group + m0) * N * N,
                            [[N, H], [N * N, hg], [H * N, 2], [1, N]]))
            return ain, apn

        def pair_view(ain, j):
            return ain[:, (2 * j) * 2 * N:(2 * j + 2) * 2 * N]

        def mm_pair(psum, lhs_tile, rhs_tile, loff=0):
            # psum[m] = M . X per matrix; lhsT blocks (k,i) at m*512+k*256+
            # i*128, rhs row-blocks k at m*512+k*256.
            for h in range(2):
                for i in range(2):
                    for k in range(2):
                        nc.tensor.matmul(
                            psum[:, h * 2 * N + i * N:h * 2 * N + (i + 1) * N],
                            lhs_tile[:, loff + h * 2 * N + k * N + i * H:
                                     loff + h * 2 * N + k * N + (i + 1) * H],
                            rhs_tile[:, h * 2 * N + k * N:
                                     h * 2 * N + (k + 1) * N],
                            start=(k == 0), stop=(k == 1))

        def mm1(apn, j):
            bp = ps_a.tile([H, 2 * 2 * N], F32, tag="psa")
            pv = pair_view(apn, j)
            mm_pair(bp, pv, pv)
            return bp

        def bq_op(bp):
            bq = bq_pool.tile([H, 2 * 2 * N], F16, tag="bq")
            nc.scalar.mul(bq[:], bp[:], -1.0 / 12.0)
            return bq

        def t1_op(ain, j, bq):
            t1 = t12_pool.tile([H, 2 * 2 * N], F16, tag="t1")
            nc.vector.tensor_tensor(t1[:], pair_view(ain, j), bq[:], op=add)
            return t1

        def x2_op(bq, ain, j):
            # 0.6*Bq via single-src tensor_scalar (4x perf mode), then a
            # plain TT (2x mode) - together ~1000ns vs a 1x-only STT ~1210ns.
            b6 = x2_pool.tile([H, 2 * 2 * N], F16, tag="b6")
            nc.vector.tensor_scalar_mul(b6[:], bq[:], 0.6)
            x2 = x2_pool.tile([H, 2 * 2 * N], F16, tag="x2")
            nc.vector.tensor_tensor(x2[:], b6[:], pair_view(ain, j), op=add)
            return x2

        def mm2(bq, t1):
            up = ps_b.tile([H, 2 * 2 * N], F32, tag="psb")
            mm_pair(up, bq, t1)
            return up

        def t2_op(up, x2):
            t2 = t12_pool.tile([H, 2 * 2 * N], F16, tag="t2")
            nc.vector.scalar_tensor_tensor(
                t2[:], up[:], 0.15, x2[:], op0=mult, op1=add)
            return t2

        def mm3(bq, t2):
            vp = ps_a.tile([H, 2 * 2 * N], F32, tag="psa")
            mm_pair(vp, bq, t2)
            return vp

        def out_copy(vp):
            wout = out_pool.tile([H, 2 * 2 * N], F16, tag="wout")
            nc.scalar.mul(wout[:], vp[:], 0.25)
            return wout

        def store_pair(p, wout):
            # issued from the (otherwise idle) GPSIMD queue to keep the
            # Sync engine's descriptor-generation budget for input loads
            nc.gpsimd.dma_start(
                bass.AP(out, 2 * p * N * N,
                        [[N, H], [N * N, 2], [H * N, 2], [1, N]]),
                wout[:])

        # ---- software-pipelined emission ----
        ain_t, apn_t = {}, {}
        bq_t, t1_t, x2_t, t2_t = {}, {}, {}, {}
        bp_t, up_t, vp_t, wout_t = {}, {}, {}, {}

        ain_t[0], apn_t[0] = load_group(0)
        if n_groups > 1:
            ain_t[1], apn_t[1] = load_group(1)

        for s in range(n_pairs + 12):
            g = s // PPG
            if s % PPG == 0 and g + 2 < n_groups:
                ain_t[g + 2], apn_t[g + 2] = load_group(g + 2)
            if s == 3:
                # keep the PE busy through the fill-cascade hole (mm2(0)
                # waits the cold mm1(0)->Bq->T1 chain ~2us): more warm
                # matmuls, before mm2(0) in the stream. Also keeps the
                # HAM activity window hot. Zero steady-state cost.
                for _ in range(5):
                    nc.tensor.matmul(warm[:, :2 * N], wz[:, :H],
                                     wz[:, :2 * N], start=True, stop=True)
            p = s - 1  # mm1
            if 0 <= p < n_pairs:
                gp, j = divmod(p, PPG)
                bp_t[p] = mm1(apn_t[gp], j)
                if j == PPG - 1:
                    apn_t.pop(gp, None)
            p = s - 3  # mm2
            if 0 <= p < n_pairs:
                up_t[p] = mm2(bq_t[p], t1_t.pop(p))
            p = s - 5  # mm3
            if 0 <= p < n_pairs:
                vp_t[p] = mm3(bq_t.pop(p), t2_t.pop(p))
            p = s - 6  # out copy (ACT) - before Bq so vp's slot frees early
            if 0 <= p < n_pairs:
                wout_t[p] = out_copy(vp_t.pop(p))
            p = s - 1  # Bq (ACT)
            if 0 <= p < n_pairs:
                bq_t[p] = bq_op(bp_t.pop(p))
            p = s - 4  # T2 (DVE) - first on DVE so up's slot frees early
            if 0 <= p < n_pairs:
                t2_t[p] = t2_op(up_t.pop(p), x2_t.pop(p))
            p = s - 2  # T1, X2 (DVE)
            if 0 <= p < n_pairs:
                gp, j = divmod(p, PPG)
                t1_t[p] = t1_op(ain_t[gp], j, bq_t[p])
                x2_t[p] = x2_op(bq_t[p], ain_t[gp], j)
                if j == PPG - 1:
                    ain_t.pop(gp, None)
            # pair store, two steps after its out_copy
            p = s - 8
            if 0 <= p < n_pairs:
                store_pair(p, wout_t.pop(p))

    if split_waits:
        _split_multi_waits(nc)
    return nc


_NC_CACHE = {}


def _prep_input(w: np.ndarray):
    """Aid = (w - w^T) + 6I and Ap = w - w^T, fp16 (skew diag exactly 0)."""
    ap = w - np.swapaxes(w, -1, -2)
    ap16 = np.ascontiguousarray(ap.astype(np.float16))
    idx = np.arange(N)
    ap[:, idx, idx] = 6.0
    aid16 = np.ascontiguousarray(ap.astype(np.float16))
    return aid16, ap16


def _postprocess(raw: np.ndarray, w: np.ndarray) -> np.ndarray:
    """res = raw (device series terms, fp16) + (w - w^T)/2 + I, in fp32."""
    res = raw.astype(np.float32)
    res += (w - np.swapaxes(w, -1, -2)) * 0.5
    idx = np.arange(N)
    res[:, idx, idx] += 1.0
    return res


def kernel(w: np.ndarray) -> np.ndarray:
    w = np.ascontiguousarray(np.asarray(w, dtype=np.float32))
    n_total = w.shape[0]
    assert w.shape == (n_total, N, N)
    per = n_total // N_CORES
    if per not in _NC_CACHE:
        _NC_CACHE[per] = _build_kernel(n_mat=per)
    nc = _NC_CACHE[per]
    aid16, ap16 = _prep_input(w)
    in_maps = [{"a": aid16[i * per:(i + 1) * per],
                "b": ap16[i * per:(i + 1) * per]} for i in range(N_CORES)]
    res = run_bass_kernel_spmd(nc, in_maps, core_ids=list(range(N_CORES)))
    raw = np.concatenate(
        [np.asarray(r["out"]).astype(np.float32) for r in res.results],
        axis=0)
    return _postprocess(raw, w)


# revision 15
# speedup vs baseline: 1.3441x; 1.0092x over previous
"""Trainium2 Bass kernel: batched truncated matrix exponential of
skew-symmetrized 256x256 matrices (nn_BatchedExponentialOrthogonalization).

Full input:  w   [512, 256, 256] fp32
Full output: out [512, 256, 256] fp32
  a = (w - w^T)/2 per matrix;  out = I + a + a^2/2! + ... + a^6/6!

Sharding: leading batch dim split across 8 NeuronCores (64 matrices each),
fully data-parallel (SPMD, same NEFF, different slabs).

Math (Horner in a^2; exactly 3 matmuls of 256^3 per matrix, the PE minimum
for a degree-6 polynomial, and NO extra identity-patch matmuls).
The host sends Aid = (w - w^T) + 6I and Ap = w - w^T in fp16 (the skew
diag is exactly 0, so Aid's diagonal is exactly 6.0).  With s = a^2:
  P1  = Ap^T Ap = -4s                  (mm1; Ap^T = -Ap)
  Bq  = -P1/12 = s/3                   (ACT psum->SBUF scaled copy)
  T1  = Aid + Bq = 2a + 6I + s/3       (tensor_tensor, DVE)
  U   = Bq . T1 = (2/3)sa + 2s + s^2/9 (mm2; Bq symmetric)
  X2  = 0.6*Bq + Aid = 0.2s + 2a + 6I  (DVE: 4x tensor_scalar + 2x tt)
  T2  = 0.15*U + X2
      = s^2/60 + 0.1 sa + 0.5 s + 2a + 6I   (DVE stt, psum in0)
  V   = Bq . T2                        (mm3)
  out = V/4 = s/2 + sa/6 + s^2/24 + s^2 a/120 + s^3/720   (ACT, fp16)
All coefficients are EXACT (no a- or I-contamination in out); the host
adds back I + a in fp32: res = out + (w - w^T)/2 (+1 on the diag).

Precision: fp16 operands (psum fp32), fp16 output; I + a exact fp32 on
host.  Rel err ~5e-4 (tolerance 2e-2).

Engines per pair-step (2 matrices): PE 24 matmuls (the minimum); DVE
T1/X2/T2 (GPSIMD fully idle - it shares SBUF ports with the DVE and
any concurrent GPSIMD op ~2.5x-slows DVE ops, so the diag-zeroed Ap
comes from the host instead of an on-device affine_select); ACT Bq +
V/4; DMA fp16 in (Aid + Ap) + fp16 out (no XBAR transpose).
Emission order inside a stage keeps the PSUM WARs loose: ACT does the
V/4 copy before Bq (frees vp's slot), DVE does T2 first (frees up's
slot before the same-stage mm3 rotates into it).  PSUM is split into
{bp,vp} / {warm,up} pools of 2 bufs each (8 banks exactly).  Stores go
per-pair from the GPSIMD queue; input b streams before a with group 0
quartered, overlapping the PE warm-up matmuls.
"""
from contextlib import ExitStack

import numpy as np

import concourse.bass as bass
import concourse.mybir as mybir
import concourse.tile as tile
from concourse.bass_utils import run_bass_kernel_spmd

F32 = mybir.dt.float32
F16 = mybir.dt.float16
N = 256
H = 128
N_CORES = 8
N_MAT_PER_CORE = 64
GROUP = 8
_MAX_WAITS = 1

N_WARM = 6


def _split_multi_waits(nc, max_waits=_MAX_WAITS):
    """This container's walrus accepts at most one sync wait per
    instruction; move excess waits onto no-fuse NOPs inserted immediately
    before, on the same engine (semantically identical - engines execute
    their stream serially)."""
    for f in nc.m.functions:
        for b in f.blocks:
            insts = b.instructions
            if not any(
                i.sync_info and i.sync_info.on_wait
                and len(i.sync_info.on_wait) > max_waits
                for i in insts
            ):
                continue
            new = []
            for inst in insts:
                si = inst.sync_info
                if si and si.on_wait and len(si.on_wait) > max_waits:
                    waits = list(si.on_wait)
                    extra, keep = waits[:-max_waits], waits[-max_waits:]
                    for k in range(0, len(extra), max_waits):
                        nop = mybir.InstNoOp(
                            name=f"I-waitsplit-{nc.next_id()}", ins=[], outs=[])
                        nop.engine = inst.engine
                        nop.bass_nofuse = True
                        nop.sync_info = mybir.SyncInfo(
                            on_wait=extra[k:k + max_waits], on_update=[])
                        new.append(nop)
                    inst.sync_info = mybir.SyncInfo(
                        on_wait=keep, on_update=list(si.on_update or []))
                new.append(inst)
            insts.clear()
            insts.extend(new)


def _build_kernel(n_mat=N_MAT_PER_CORE, group=GROUP, split_waits=True):
    nc = bass.Bass(trn_type="TRN2")
    a_in = nc.dram_tensor("a", [n_mat, N, N], F16, kind="ExternalInput")
    b_in = nc.dram_tensor("b", [n_mat, N, N], F16, kind="ExternalInput")
    out = nc.dram_tensor("out", [n_mat, N, N], F16, kind="ExternalOutput")
    n_groups = n_mat // group
    n_pairs = n_mat // 2
    PPG = group // 2  # pairs per group

    mult = mybir.AluOpType.mult
    add = mybir.AluOpType.add

    with ExitStack() as ctx:
        tc = ctx.enter_context(tile.TileContext(nc))
        const_pool = ctx.enter_context(tc.tile_pool(name="const", bufs=1))
        io_pool = ctx.enter_context(tc.tile_pool(name="io", bufs=8))
        bq_pool = ctx.enter_context(tc.tile_pool(name="bq", bufs=7))
        t12_pool = ctx.enter_context(tc.tile_pool(name="t12", bufs=8))
        x2_pool = ctx.enter_context(tc.tile_pool(name="x2", bufs=6))
        out_pool = ctx.enter_context(tc.tile_pool(name="outp", bufs=3))
        # PSUM split so every WAR clears >= a full stage (or an early-ACT
        # op) before the PE rotates into the slot: bp+vp alternate in ps_a
        # (bp: slot freed by the Bq ACT copy one stage earlier; vp: freed
        # by the V/4 copy that runs FIRST in the ACT stream of the same
        # stage, ~1.1us before mm3 needs it), warm+up rotate in ps_b (up's
        # slot freed by T2, a full stage earlier).
        ps_a = ctx.enter_context(
            tc.tile_pool(name="psa", bufs=2, space="PSUM"))
        ps_b = ctx.enter_context(
            tc.tile_pool(name="psb", bufs=2, space="PSUM"))

        # ---- PE p-state warm-up + ACT table preload during first DMA ----
        wz = const_pool.tile([H, 2 * N], F16, tag="wz")
        nc.vector.memset(wz[:], 0.0)
        warm = ps_b.tile([H, 2 * 2 * N], F32, tag="psb")
        for _ in range(N_WARM):
            nc.tensor.matmul(warm[:, :2 * N], wz[:, :H], wz[:, :2 * N],
                             start=True, stop=True)
        warm_sb = const_pool.tile([H, 8], F32, tag="warmsb")
        nc.scalar.copy(warm_sb[:], warm[:, 0:8])

        def load_group(g):
            # b (mm1's operand) is issued before a; group 0's b arrives in
            # pair-sized quarters so mm1(0) can start as early as possible.
            ain = io_pool.tile([H, group * 2 * N], F16, tag="ain")
            apn = io_pool.tile([H, group * 2 * N], F16, tag="apn")
            hg = group // 2
            nb = 4 if g == 0 else 2
            cb = group // nb
            for chunk in range(nb):
                m0 = chunk * cb
                nc.sync.dma_start(
                    apn[:, m0 * 2 * N:(m0 + cb) * 2 * N],
                    bass.AP(b_in, (g * group + m0) * N * N,
                            [[N, H], [N * N, cb], [H * N, 2], [1, N]]))
            for half in range(2):
                m0 = half * hg
                nc.sync.dma_start(
                    ain[:, m0 * 2 * N:(m0 + hg) * 2 * N],
                    bass.AP(a_in, (g * group + m0) * N * N,
                            [[N, H], [N * N, hg], [H * N, 2], [1, N]]))
            return ain, apn

        def pair_view(ain, j):
            return ain[:, (2 * j) * 2 * N:(2 * j + 2) * 2 * N]

        def mm_pair(psum, lhs_tile, rhs_tile, loff=0):
            # psum[m] = M . X per matrix; lhsT blocks (k,i) at m*512+k*256+
            # i*128, rhs row-blocks k at m*512+k*256.
            for h in range(2):
                for i in range(2):
                    for k in range(2):
                        nc.tensor.matmul(
                            psum[:, h * 2 * N + i * N:h * 2 * N + (i + 1) * N],
                            lhs_tile[:, loff + h * 2 * N + k * N + i * H:
                                     loff + h * 2 * N + k * N + (i + 1) * H],
                            rhs_tile[:, h * 2 * N + k * N:
                                     h * 2 * N + (k + 1) * N],
                            start=(k == 0), stop=(k == 1))

        def mm1(apn, j):
            bp = ps_a.tile([H, 2 * 2 * N], F32, tag="psa")
            pv = pair_view(apn, j)
            mm_pair(bp, pv, pv)
            return bp

        def bq_op(bp, split=False):
            bq = bq_pool.tile([H, 2 * 2 * N], F16, tag="bq")
            if split:
                # pair 0 only: halves, so mm2(0) can start off the h0
                # region while h1 is still being copied (fill cascade)
                nc.scalar.mul(bq[:, :2 * N], bp[:, :2 * N], -1.0 / 12.0)
                nc.scalar.mul(bq[:, 2 * N:], bp[:, 2 * N:], -1.0 / 12.0)
            else:
                nc.scalar.mul(bq[:], bp[:], -1.0 / 12.0)
            return bq

        def t1_op(ain, j, bq, split=False):
            t1 = t12_pool.tile([H, 2 * 2 * N], F16, tag="t1")
            pv = pair_view(ain, j)
            if split:
                nc.vector.tensor_tensor(
                    t1[:, :2 * N], pv[:, :2 * N], bq[:, :2 * N], op=add)
                nc.vector.tensor_tensor(
                    t1[:, 2 * N:], pv[:, 2 * N:], bq[:, 2 * N:], op=add)
            else:
                nc.vector.tensor_tensor(t1[:], pv, bq[:], op=add)
            return t1

        def x2_op(bq, ain, j):
            # 0.6*Bq via single-src tensor_scalar (4x perf mode), then a
            # plain TT (2x mode) - together ~1000ns vs a 1x-only STT ~1210ns.
            b6 = x2_pool.tile([H, 2 * 2 * N], F16, tag="b6")
            nc.vector.tensor_scalar_mul(b6[:], bq[:], 0.6)
            x2 = x2_pool.tile([H, 2 * 2 * N], F16, tag="x2")
            nc.vector.tensor_tensor(x2[:], b6[:], pair_view(ain, j), op=add)
            return x2

        def mm2(bq, t1):
            up = ps_b.tile([H, 2 * 2 * N], F32, tag="psb")
            mm_pair(up, bq, t1)
            return up

        def t2_op(up, x2):
            t2 = t12_pool.tile([H, 2 * 2 * N], F16, tag="t2")
            nc.vector.scalar_tensor_tensor(
                t2[:], up[:], 0.15, x2[:], op0=mult, op1=add)
            return t2

        def mm3(bq, t2):
            vp = ps_a.tile([H, 2 * 2 * N], F32, tag="psa")
            mm_pair(vp, bq, t2)
            return vp

        def out_copy(vp):
            wout = out_pool.tile([H, 2 * 2 * N], F16, tag="wout")
            nc.scalar.mul(wout[:], vp[:], 0.25)
            return wout

        def store_pair(p, wout):
            # issued from the (otherwise idle) GPSIMD queue to keep the
            # Sync engine's descriptor-generation budget for input loads
            nc.gpsimd.dma_start(
                bass.AP(out, 2 * p * N * N,
                        [[N, H], [N * N, 2], [H * N, 2], [1, N]]),
                wout[:])

        # ---- software-pipelined emission ----
        ain_t, apn_t = {}, {}
        bq_t, t1_t, x2_t, t2_t = {}, {}, {}, {}
        bp_t, up_t, vp_t, wout_t = {}, {}, {}, {}

        ain_t[0], apn_t[0] = load_group(0)
        if n_groups > 1:
            ain_t[1], apn_t[1] = load_group(1)

        for s in range(n_pairs + 12):
            g = s // PPG
            if s % PPG == 0 and g + 2 < n_groups:
                ain_t[g + 2], apn_t[g + 2] = load_group(g + 2)
            if s == 3:
                # keep the PE busy through the fill-cascade hole (mm2(0)
                # waits the cold mm1(0)->Bq->T1 chain ~2us): more warm
                # matmuls, before mm2(0) in the stream. Also keeps the
                # HAM activity window hot. Zero steady-state cost.
                for _ in range(9):
                    nc.tensor.matmul(warm[:, :2 * N], wz[:, :H],
                                     wz[:, :2 * N], start=True, stop=True)
            p = s - 1  # mm1
            if 0 <= p < n_pairs:
                gp, j = divmod(p, PPG)
                bp_t[p] = mm1(apn_t[gp], j)
                if j == PPG - 1:
                    apn_t.pop(gp, None)
            p = s - 3  # mm2
            if 0 <= p < n_pairs:
                up_t[p] = mm2(bq_t[p], t1_t.pop(p))
            p = s - 5  # mm3
            if 0 <= p < n_pairs:
                vp_t[p] = mm3(bq_t.pop(p), t2_t.pop(p))
            p = s - 6  # out copy (ACT) - before Bq so vp's slot frees early
            if 0 <= p < n_pairs:
                wout_t[p] = out_copy(vp_t.pop(p))
            p = s - 1  # Bq (ACT)
            if 0 <= p < n_pairs:
                bq_t[p] = bq_op(bp_t.pop(p), split=(p == 0))
            p = s - 4  # T2 (DVE) - first on DVE so up's slot frees early
            if 0 <= p < n_pairs:
                t2_t[p] = t2_op(up_t.pop(p), x2_t.pop(p))
            p = s - 2  # T1, X2 (DVE)
            if 0 <= p < n_pairs:
                gp, j = divmod(p, PPG)
                t1_t[p] = t1_op(ain_t[gp], j, bq_t[p], split=(p == 0))
                x2_t[p] = x2_op(bq_t[p], ain_t[gp], j)
                if j == PPG - 1:
                    ain_t.pop(gp, None)
            # pair store, two steps after its out_copy
            p = s - 8
            if 0 <= p < n_pairs:
                store_pair(p, wout_t.pop(p))

    if split_waits:
        _split_multi_waits(nc)
    return nc


_NC_CACHE = {}


def _prep_input(w: np.ndarray):
    """Aid = (w - w^T) + 6I and Ap = w - w^T, fp16 (skew diag exactly 0)."""
    ap = w - np.swapaxes(w, -1, -2)
    ap16 = np.ascontiguousarray(ap.astype(np.float16))
    idx = np.arange(N)
    ap[:, idx, idx] = 6.0
    aid16 = np.ascontiguousarray(ap.astype(np.float16))
    return aid16, ap16


def _postprocess(raw: np.ndarray, w: np.ndarray) -> np.ndarray:
    """res = raw (device series terms, fp16) + (w - w^T)/2 + I, in fp32."""
    res = raw.astype(np.float32)
    res += (w - np.swapaxes(w, -1, -2)) * 0.5
    idx = np.arange(N)
    res[:, idx, idx] += 1.0
    return res


def kernel(w: np.ndarray) -> np.ndarray:
    w = np.ascontiguousarray(np.asarray(w, dtype=np.float32))
    n_total = w.shape[0]
    assert w.shape == (n_total, N, N)
    per = n_total // N_CORES
    if per not in _NC_CACHE:
        _NC_CACHE[per] = _build_kernel(n_mat=per)
    nc = _NC_CACHE[per]
    aid16, ap16 = _prep_input(w)
    in_maps = [{"a": aid16[i * per:(i + 1) * per],
                "b": ap16[i * per:(i + 1) * per]} for i in range(N_CORES)]
    res = run_bass_kernel_spmd(nc, in_maps, core_ids=list(range(N_CORES)))
    raw = np.concatenate(
        [np.asarray(r["out"]).astype(np.float32) for r in res.results],
        axis=0)
    return _postprocess(raw, w)
